# revision 1
# baseline (speedup 1.0000x reference)
"""Trainium2 Bass kernel for efficient-attention (nn_Attention_65532611003000).

Sharding: data-parallel over batch. B == n_cores == 8, so core i processes
batch element i end-to-end; no collectives are needed.

Per-core math ([Nt, Ch] = [4096, 512] activations, H=8 heads, 64 ch/head):
  khat = exp((input_+y) @ Wk)            # bk drops out: softmax over tokens
                                         # is invariant to a per-column shift
  val0 = input_ @ Wv                     # bv folded into ctx (softmax cols
                                         # sum to 1 -> ctx += bv)
  S_t  = sum_chunks khat_t^T @ [val0_t | 1]   # ones col accumulates Zk
  ctx  = S / Zk + bv                     # per head: [64, 64]
  qhat = exp(y @ Wq + bq); qn = qhat / rowsum_per_head(qhat)
  attT = ctx^T @ qnT                     # channel-major, per head
  out  = att @ Wr + br

"""

import sys

sys.path.insert(0, "/opt/trn_rl_repo")

import numpy as np
import ml_dtypes
from contextlib import ExitStack

import concourse.bass as bass
import concourse.bacc as bacc
import concourse.mybir as mybir
import concourse.tile as tile
from concourse.bass_utils import run_bass_kernel_spmd

B, Nt, Ch = 8, 4096, 512
H, HK = 8, 64
P = 128            # token chunk rows / SBUF partitions
NT = Nt // P       # 32 token chunks
CT = Ch // P       # 4 contraction tiles
GRP = 4            # pass-2 chunks per group (512 tokens)
NG = NT // GRP

F32 = mybir.dt.float32
F32R = mybir.dt.float32r
BF16 = mybir.dt.bfloat16
AX = mybir.AxisListType
AF = mybir.ActivationFunctionType

BF16_NP = ml_dtypes.bfloat16


def build_nc(debug=False):
    nc = bacc.Bacc(None)

    inp_d = nc.declare_dram_parameter("input_", [Nt, Ch], F32, isOutput=False)
    y_d = nc.declare_dram_parameter("y", [Nt, Ch], F32, isOutput=False)
    wk_d = nc.declare_dram_parameter("Wk_r", [P, CT * Ch], F32, isOutput=False)
    wq_d = nc.declare_dram_parameter("Wq_r", [P, CT * Ch], F32, isOutput=False)
    wv_d = nc.declare_dram_parameter("Wv_r", [P, CT * Ch], F32, isOutput=False)
    wr_d = nc.declare_dram_parameter("Wr_r", [P, CT * Ch], F32, isOutput=False)
    bq_d = nc.declare_dram_parameter("bq_row", [1, Ch], BF16, isOutput=False)
    brb_d = nc.declare_dram_parameter("br_bcast", [P, Ch], F32, isOutput=False)
    bvb_d = nc.declare_dram_parameter("bv_blk", [P, Ch], BF16, isOutput=False)
    id32_d = nc.declare_dram_parameter("ident32", [P, P], F32, isOutput=False)
    id16_d = nc.declare_dram_parameter("ident16", [P, P], BF16, isOutput=False)
    ones_d = nc.declare_dram_parameter("ones_row", [1, P], BF16, isOutput=False)
    onescol_d = nc.declare_dram_parameter(
        "ones_col", [P, CT * 2], BF16, isOutput=False
    )
    out_d = nc.declare_dram_parameter("out", [Nt, Ch], F32, isOutput=True)
    if debug:
        dbg = {
            "dbg_xT": nc.declare_dram_parameter("dbg_xT", [P, Ch], F32, isOutput=True),
            "dbg_xsT": nc.declare_dram_parameter("dbg_xsT", [P, Ch], F32, isOutput=True),
            "dbg_khat": nc.declare_dram_parameter("dbg_khat", [P, Ch], BF16, isOutput=True),
            "dbg_vaug": nc.declare_dram_parameter("dbg_vaug", [P, CT * 130], BF16, isOutput=True),
            "dbg_s": nc.declare_dram_parameter("dbg_s", [P, CT * 130], F32, isOutput=True),
            "dbg_zkinv": nc.declare_dram_parameter("dbg_zkinv", [P, CT], F32, isOutput=True),
            "dbg_ctx": nc.declare_dram_parameter("dbg_ctx", [P, CT * P], BF16, isOutput=True),
            "dbg_qhat": nc.declare_dram_parameter("dbg_qhat", [P, Ch], F32, isOutput=True),
            "dbg_rs": nc.declare_dram_parameter("dbg_rs", [P, 2 * H], F32, isOutput=True),
            "dbg_qn": nc.declare_dram_parameter("dbg_qn", [P, Ch], F32, isOutput=True),
            "dbg_qnT": nc.declare_dram_parameter("dbg_qnT", [P, CT * GRP * P], BF16, isOutput=True),
            "dbg_attT": nc.declare_dram_parameter("dbg_attT", [P, CT * GRP * P], F32, isOutput=True),
        }

    with tile.TileContext(nc) as tc, ExitStack() as ctx:
        const = ctx.enter_context(tc.tile_pool(name="const", bufs=1))

        wk = const.tile([P, CT, Ch], F32R)
        wq = const.tile([P, CT, Ch], F32R)
        wv = const.tile([P, CT, Ch], F32R)
        wr = const.tile([P, CT, Ch], F32R)
        w_raw = [
            const.tile([P, CT, Ch], F32, name=f"wraw{n}", tag=f"wraw{n}")
            for n in range(4)
        ]
        bq = const.tile([1, Ch], BF16)
        brb = const.tile([P, Ch], F32)
        bvb = const.tile([P, Ch], BF16)
        id32 = const.tile([P, P], F32)
        id16 = const.tile([P, P], BF16)
        ones1 = const.tile([1, P], BF16)
        yT_all = const.tile([P, NT, CT, P], F32R)     # resident y^T, 64KB/part
        ctxR = const.tile([P, CT, P], BF16)           # per-head ctx, blockdiag
        zkinv = const.tile([P, CT], F32)

        for n, (t_sb, t_d) in enumerate(
            ((wk, wk_d), (wq, wq_d), (wv, wv_d), (wr, wr_d))
        ):
            nc.sync.dma_start(
                w_raw[n][:], t_d[:].rearrange("p (t j) -> p t j", t=CT)
            )
            # rounding copy into the fp32r weight tile
            if n % 2 == 0:
                nc.vector.tensor_copy(t_sb[:], w_raw[n][:])
            else:
                nc.scalar.copy(t_sb[:], w_raw[n][:])
        nc.sync.dma_start(bq[:], bq_d[:])
        nc.sync.dma_start(id32[:], id32_d[:])
        nc.sync.dma_start(brb[:], brb_d[:])
        nc.sync.dma_start(bvb[:], bvb_d[:])
        nc.sync.dma_start(id16[:], id16_d[:])
        nc.sync.dma_start(ones1[:], ones_d[:])

        # ---------------- pass 1: khat, v, S & Zk accumulation --------------
        with (
            tc.tile_pool(name="io1", bufs=3) as io1,
            tc.tile_pool(name="sb1", bufs=2) as sb1,
            tc.tile_pool(name="ps_t", bufs=2, space="PSUM") as ps_t,
            tc.tile_pool(name="ps_k", bufs=1, space="PSUM") as ps_k,
            tc.tile_pool(name="ps_v", bufs=1, space="PSUM") as ps_v,
            tc.tile_pool(name="ps_s", bufs=1, space="PSUM") as ps_s,
        ):
            s_acc = [
                ps_s.tile([P, 130], F32, tag=f"sacc{t}", name=f"sacc{t}")
                for t in range(CT)
            ]
            # manually double-buffered [val0 | ones] tiles; ones cols written once
            v_aug_bufs = [
                sb1.tile([P, CT, 130], BF16, tag=f"vaug{n}", name=f"vaug{n}")
                for n in range(2)
            ]
            for n in range(2):
                nc.sync.dma_start(
                    v_aug_bufs[n][:, :, 128:130],
                    onescol_d[:].rearrange("p (t c) -> p t c", t=CT),
                )

            for i in range(NT):
                x_in = io1.tile([P, Ch], F32, tag="xin")
                y_in = io1.tile([P, Ch], F32, tag="yin")
                nc.sync.dma_start(x_in[:], inp_d[P * i : P * (i + 1), :])
                nc.sync.dma_start(y_in[:], y_d[P * i : P * (i + 1), :])

                xT_ps = ps_t.tile([P, Ch], F32, tag="tp")
                for t in range(CT):
                    nc.tensor.transpose(
                        xT_ps[:, P * t : P * (t + 1)],
                        x_in[:, P * t : P * (t + 1)],
                        id32[:],
                    )
                xT = sb1.tile([P, Ch], F32R, tag="xT")
                nc.vector.tensor_copy(xT[:], xT_ps[:])

                yT_ps = ps_t.tile([P, Ch], F32, tag="tp")
                for t in range(CT):
                    nc.tensor.transpose(
                        yT_ps[:, P * t : P * (t + 1)],
                        y_in[:, P * t : P * (t + 1)],
                        id32[:],
                    )
                nc.vector.tensor_copy(
                    yT_all[:, i, :, :],
                    yT_ps[:].rearrange("p (t q) -> p t q", t=CT),
                )

                xsT = sb1.tile([P, Ch], F32R, tag="xsT")
                nc.vector.tensor_add(
                    xsT[:].rearrange("p (t q) -> p t q", t=CT),
                    xT[:].rearrange("p (t q) -> p t q", t=CT),
                    yT_all[:, i, :, :],
                )

                kpre = ps_k.tile([P, Ch], F32, tag="kpre")
                for t in range(CT):
                    nc.tensor.matmul(
                        kpre[:],
                        xsT[:, P * t : P * (t + 1)],
                        wk[:, t, :],
                        start=(t == 0),
                        stop=(t == CT - 1),
                    )
                khat = sb1.tile([P, Ch], BF16, tag="khat")
                nc.scalar.activation(khat[:], kpre[:], AF.Exp)

                vpre = ps_v.tile([P, Ch], F32, tag="vpre")
                for t in range(CT):
                    nc.tensor.matmul(
                        vpre[:],
                        xT[:, P * t : P * (t + 1)],
                        wv[:, t, :],
                        start=(t == 0),
                        stop=(t == CT - 1),
                    )
                v_aug = v_aug_bufs[i % 2]
                nc.scalar.copy(
                    v_aug[:, :, 0:128],
                    vpre[:].rearrange("p (t q) -> p t q", t=CT),
                )

                for t in range(CT):
                    nc.tensor.matmul(
                        s_acc[t][:],
                        khat[:, P * t : P * (t + 1)],
                        v_aug[:, t, :],
                        start=(i == 0),
                        stop=(i == NT - 1),
                    )

                if debug and i == 0:
                    nc.sync.dma_start(dbg["dbg_xT"][:], xT[:].bitcast(F32))
                    nc.sync.dma_start(dbg["dbg_xsT"][:], xsT[:].bitcast(F32))
                    nc.sync.dma_start(dbg["dbg_khat"][:], khat[:])
                    nc.sync.dma_start(
                        dbg["dbg_vaug"][:].rearrange("p (t c) -> p t c", t=CT),
                        v_aug[:],
                    )

            # ------------- epilogue: ctx = S * zkinv + bv ------------------
            for t in range(CT):
                nc.vector.reciprocal(zkinv[:, t : t + 1], s_acc[t][:, 128:129])
            for t in range(CT):
                nc.vector.tensor_copy(ctxR[:, t, :], bvb[:, P * t : P * (t + 1)])
                for blk in range(2):
                    p0 = 64 * blk
                    nc.vector.scalar_tensor_tensor(
                        ctxR[p0 : p0 + 64, t, p0 : p0 + 64],
                        s_acc[t][p0 : p0 + 64, p0 : p0 + 64],
                        zkinv[p0 : p0 + 64, t : t + 1],
                        bvb[p0 : p0 + 64, P * t + p0 : P * t + p0 + 64],
                        op0=mybir.AluOpType.mult,
                        op1=mybir.AluOpType.add,
                    )
            if debug:
                s_dump = sb1.tile([P, CT, 130], F32, name="s_dump", tag="s_dump")
                for t in range(CT):
                    nc.vector.tensor_copy(s_dump[:, t, :], s_acc[t][:])
                nc.sync.dma_start(
                    dbg["dbg_s"][:].rearrange("p (t c) -> p t c", t=CT), s_dump[:]
                )
                nc.sync.dma_start(dbg["dbg_zkinv"][:], zkinv[:])
                nc.sync.dma_start(
                    dbg["dbg_ctx"][:].rearrange("p (t c) -> p t c", t=CT), ctxR[:]
                )

        # ---------------- pass 2: q softmax, attend, reproject ---------------
        with (
            tc.tile_pool(name="io2", bufs=3) as io2,
            tc.tile_pool(name="sb2", bufs=2) as sb2,
            tc.tile_pool(name="ps_q", bufs=2, space="PSUM") as ps_q,
            tc.tile_pool(name="ps_qt", bufs=2, space="PSUM") as ps_qt,
            tc.tile_pool(name="ps_a", bufs=2, space="PSUM") as ps_a,
            tc.tile_pool(name="ps_o", bufs=2, space="PSUM") as ps_o,
        ):
            for g in range(NG):
                qnT = sb2.tile([P, CT, GRP, P], BF16, tag="qnT")
                for j in range(GRP):
                    i = g * GRP + j
                    qpre = ps_q.tile([P, Ch], F32, tag="qpre")
                    for t in range(CT):
                        nc.tensor.matmul(
                            qpre[:],
                            yT_all[:, i, t, :],
                            wq[:, t, :],
                            start=(t == 0),
                            stop=False,
                        )
                    nc.tensor.matmul(
                        qpre[:], ones1[:], bq[:], start=False, stop=True
                    )
                    qhat = sb2.tile([P, Ch], F32, tag="qhat")
                    nc.scalar.activation(qhat[:], qpre[:], AF.Exp)
                    rs = sb2.tile([P, H, 1], F32, tag="rs")
                    nc.vector.reduce_sum(
                        rs[:, :, 0],
                        qhat[:].rearrange("p (h k) -> p h k", h=H),
                        axis=AX.X,
                    )
                    rinv = sb2.tile([P, H, 1], F32, tag="rinv")
                    nc.vector.reciprocal(rinv[:], rs[:])
                    qn = sb2.tile([P, Ch], F32, tag="qn")
                    nc.vector.tensor_mul(
                        qn[:].rearrange("p (h k) -> p h k", h=H),
                        qhat[:].rearrange("p (h k) -> p h k", h=H),
                        rinv[:].broadcast_to([P, H, HK]),
                    )
                    qnT_ps = ps_qt.tile([P, Ch], F32, tag="qnt")
                    for t in range(CT):
                        nc.tensor.transpose(
                            qnT_ps[:, P * t : P * (t + 1)],
                            qn[:, P * t : P * (t + 1)],
                            id32[:],
                        )
                    nc.scalar.copy(
                        qnT[:, :, j, :],
                        qnT_ps[:].rearrange("p (t q) -> p t q", t=CT),
                    )
                    if debug and i == 0:
                        nc.sync.dma_start(dbg["dbg_qhat"][:], qhat[:])
                        nc.sync.dma_start(dbg["dbg_rs"][:, 0:H], rs[:, :, 0])
                        nc.sync.dma_start(dbg["dbg_rs"][:, H : 2 * H], rinv[:, :, 0])
                        nc.sync.dma_start(dbg["dbg_qn"][:], qn[:])

                attT = sb2.tile([P, CT, GRP * P], F32R, tag="attT")
                for t in range(CT):
                    a_ps = ps_a.tile([P, GRP * P], F32, tag="aps")
                    nc.tensor.matmul(
                        a_ps[:],
                        ctxR[:, t, :],
                        qnT[:, t, :, :].rearrange("p g q -> p (g q)"),
                        start=True,
                        stop=True,
                    )
                    nc.scalar.copy(attT[:, t, :], a_ps[:])
                if debug and g == 0:
                    nc.sync.dma_start(
                        dbg["dbg_qnT"][:].rearrange("p (t g q) -> p t g q", t=CT, g=GRP),
                        qnT[:],
                    )
                    nc.sync.dma_start(
                        dbg["dbg_attT"][:].rearrange("p (t c) -> p t c", t=CT),
                        attT[:].bitcast(F32),
                    )

                for j in range(GRP):
                    i = g * GRP + j
                    opre = ps_o.tile([P, Ch], F32, tag="opre")
                    for t in range(CT):
                        nc.tensor.matmul(
                            opre[:],
                            attT[:, t, P * j : P * (j + 1)],
                            wr[:, t, :],
                            start=(t == 0),
                            stop=(t == CT - 1),
                        )
                    o_sb = io2.tile([P, Ch], F32, tag="osb")
                    nc.vector.tensor_add(o_sb[:], opre[:], brb[:])
                    nc.sync.dma_start(out_d[P * i : P * (i + 1), :], o_sb[:])

    nc.finalize()
    return nc


def _host_consts(Wk, bk, Wq, bq, Wv, bv, Wr, br):
    def rearr(w):
        return (
            np.ascontiguousarray(
                w.reshape(CT, P, Ch).transpose(1, 0, 2).reshape(P, CT * Ch)
            ).astype(np.float32)
        )

    bvb = np.zeros((P, Ch), np.float32)
    for t in range(CT):
        for blk in range(2):
            p0 = 64 * blk
            c0 = P * t + p0
            bvb[p0 : p0 + 64, c0 : c0 + 64] = bv[None, c0 : c0 + 64]
    return {
        "Wk_r": rearr(Wk),
        "Wq_r": rearr(Wq),
        "Wv_r": rearr(Wv),
        "Wr_r": rearr(Wr),
        "bq_row": np.ascontiguousarray(bq[None, :]).astype(BF16_NP),
        "br_bcast": np.ascontiguousarray(np.tile(br[None, :], (P, 1))).astype(
            np.float32
        ),
        "bv_blk": bvb.astype(BF16_NP),
        "ident32": np.eye(P, dtype=np.float32),
        "ident16": np.eye(P).astype(BF16_NP),
        "ones_row": np.ones((1, P), BF16_NP),
        "ones_col": np.ones((P, CT * 2), BF16_NP),
    }


_NC_CACHE = {}


def _get_nc():
    if "nc" not in _NC_CACHE:
        _NC_CACHE["nc"] = build_nc()
    return _NC_CACHE["nc"]


def kernel(input_, y, Wk, bk, Wq, bq, Wv, bv, Wr, br, _trace=False, _tmpdir=None):
    input_ = np.asarray(input_, np.float32)
    y = np.asarray(y, np.float32)
    consts = _host_consts(
        np.asarray(Wk, np.float32), np.asarray(bk, np.float32),
        np.asarray(Wq, np.float32), np.asarray(bq, np.float32),
        np.asarray(Wv, np.float32), np.asarray(bv, np.float32),
        np.asarray(Wr, np.float32), np.asarray(br, np.float32),
    )
    nc = _get_nc()
    in_maps = [
        {
            "input_": np.ascontiguousarray(input_[i]),
            "y": np.ascontiguousarray(y[i]),
            **consts,
        }
        for i in range(B)
    ]
    res = run_bass_kernel_spmd(
        nc, in_maps, core_ids=list(range(B)), trace=_trace, tmpdir=_tmpdir
    )
    out = np.stack([res.results[i]["out"] for i in range(B)], axis=0)
    if _trace:
        return out, res
    return out



# revision 4
# speedup vs baseline: 1.3510x; 1.3510x over previous
"""Trainium2 Bass kernel for efficient-attention (nn_Attention_65532611003000).

Sharding: data-parallel over batch. B == n_cores == 8, so core i processes
batch element i end-to-end; no collectives are needed.

v2 design (fp8 DoubleRow projections, k-major pass 2):

Per-core math ([Nt, Ch] = [4096, 512] activations, H=8 heads, 64 ch/head):
  pass 1 (per 128-token tile):
    x16, y16 = bf16(x), bf16(y)               # gpsimd casting DMA from DRAM
    xT, yT = transpose(x16), transpose(y16)   # PE transposes, bf16, 1cyc/row
    xT8, yT8 = fp8(xT), fp8(yT)               # PSUM->SBUF copies cast to fp8
    kpre = xT8'@Wk8 + yT8'@Wk8                # fp8 DoubleRow (256-deep)
    khat = bf16(exp(kpre))                    # bk drops out of token-softmax
    vpre = xT8'@Wv8                           # fp8 DoubleRow
    S_t += khat_t^T @ [vpre_t | 1]            # bf16, per 128-ch block t
  epilogue: ctx_t = S_t * (1/Zk) + bv         # blockdiag per 2 heads
  pass 2 (per 512-token group), all in k-major (channel, token) layout:
    qpreT = Wq8' @ yT8                        # fp8 DoubleRow, transposed out
    u = bf16(exp(qpreT + bq))                 # bq as per-partition bias
    den = sel' @ u                            # per-head token sums (matmul)
    denB_t = selT' @ bf16(1/den)              # broadcast to v partitions
    att_t = bf16((ctx_t^T @ u_t) * denB_t)    # unnormalized attend, then mul
    out_j = sum_t att_t[:, j]' @ Wr16_t + br  # reprojection, token-major out
"""

import sys

sys.path.insert(0, "/opt/trn_rl_repo")

import numpy as np
import ml_dtypes
from contextlib import ExitStack

import concourse.bass as bass
import concourse.bacc as bacc
import concourse.mybir as mybir
import concourse.tile as tile
from concourse.bass_utils import run_bass_kernel_spmd

B, Nt, Ch = 8, 4096, 512
H, HK = 8, 64
P = 128            # token chunk rows / SBUF partitions
NT = Nt // P       # 32 token tiles
CT = Ch // P       # 4 channel blocks
GRP = 4            # pass-2 tiles per group (512 tokens)
NG = NT // GRP     # 8 groups

F32 = mybir.dt.float32
F32R = mybir.dt.float32r
BF16 = mybir.dt.bfloat16
F8 = mybir.dt.float8e4
AX = mybir.AxisListType
AF = mybir.ActivationFunctionType
DR = mybir.MatmulPerfMode.DoubleRow

BF16_NP = ml_dtypes.bfloat16
F8_NP = ml_dtypes.float8_e4m3


def build_nc():
    nc = bacc.Bacc(None)

    inp_d = nc.declare_dram_parameter("input_", [Nt, Ch], F32, isOutput=False)
    y_d = nc.declare_dram_parameter("y", [Nt, Ch], F32, isOutput=False)
    wk_d = nc.declare_dram_parameter("wk8", [P, 4 * Ch], F8, isOutput=False)
    wv_d = nc.declare_dram_parameter("wv8", [P, 4 * Ch], F8, isOutput=False)
    wq_d = nc.declare_dram_parameter("wq8", [P, 4 * Ch], F8, isOutput=False)
    wr_d = nc.declare_dram_parameter("wr16", [P, CT * Ch], BF16, isOutput=False)
    bqc_d = nc.declare_dram_parameter("bq_col", [P, CT], F32, isOutput=False)
    seld_d = nc.declare_dram_parameter("sel_den", [P, CT * H], BF16, isOutput=False)
    selt_d = nc.declare_dram_parameter("selT_bc", [H, CT * P], BF16, isOutput=False)
    bvb_d = nc.declare_dram_parameter("bv_blk", [P, Ch], F32, isOutput=False)
    brb_d = nc.declare_dram_parameter("br_bcast", [P, Ch], F32, isOutput=False)
    id16_d = nc.declare_dram_parameter("ident16", [P, P], BF16, isOutput=False)
    ones_d = nc.declare_dram_parameter("ones_col", [P, CT * 2], BF16, isOutput=False)
    out_d = nc.declare_dram_parameter("out", [Nt, Ch], F32, isOutput=True)

    with tile.TileContext(nc) as tc, ExitStack() as ctx:
        const = ctx.enter_context(tc.tile_pool(name="const", bufs=1))

        wk8 = const.tile([P, 2, 2, Ch], F8)
        wv8 = const.tile([P, 2, 2, Ch], F8)
        wq8 = const.tile([P, 2, 2, CT, P], F8)
        wr16 = const.tile([P, CT, Ch], BF16)
        bq_col = const.tile([P, CT], F32)
        sel_den = const.tile([P, CT, H], BF16)
        selT = const.tile([H, CT, P], BF16)
        bvb = const.tile([P, Ch], F32)
        brb = const.tile([P, Ch], F32)
        id16 = const.tile([P, P], BF16)
        yT8 = const.tile([P, CT, NT, P], F8)     # resident y^T, [p, blk, tile, tok]
        ctxR = const.tile([P, CT, P], BF16)      # per-head ctx, blockdiag
        zkinv = const.tile([P, CT], F32)

        nc.sync.dma_start(wk8[:], wk_d[:].rearrange("p (g i o) -> p g i o", g=2, i=2))
        nc.sync.dma_start(wv8[:], wv_d[:].rearrange("p (g i o) -> p g i o", g=2, i=2))
        nc.sync.dma_start(
            wq8[:], wq_d[:].rearrange("p (g i kb m) -> p g i kb m", g=2, i=2, kb=CT)
        )
        nc.sync.dma_start(wr16[:], wr_d[:].rearrange("p (t o) -> p t o", t=CT))
        nc.sync.dma_start(bq_col[:], bqc_d[:])
        nc.sync.dma_start(sel_den[:], seld_d[:].rearrange("p (t h) -> p t h", t=CT))
        nc.sync.dma_start(selT[:], selt_d[:].rearrange("p (t m) -> p t m", t=CT))
        nc.sync.dma_start(bvb[:], bvb_d[:])
        nc.sync.dma_start(brb[:], brb_d[:])
        nc.sync.dma_start(id16[:], id16_d[:])

        # ---------------- pass 1: khat, v, S & Zk accumulation --------------
        with (
            tc.tile_pool(name="io1", bufs=3) as io1,
            tc.tile_pool(name="sb1", bufs=2) as sb1,
            tc.tile_pool(name="ps_tp", bufs=2, space="PSUM") as ps_tp,
            tc.tile_pool(name="ps_k", bufs=1, space="PSUM") as ps_k,
            tc.tile_pool(name="ps_v", bufs=1, space="PSUM") as ps_v,
            tc.tile_pool(name="ps_s", bufs=1, space="PSUM") as ps_s,
        ):
            s_acc = [
                ps_s.tile([P, 130], F32, tag=f"sacc{t}", name=f"sacc{t}")
                for t in range(CT)
            ]
            v_aug_bufs = [
                sb1.tile([P, CT, 130], BF16, tag=f"vaug{n}", name=f"vaug{n}")
                for n in range(2)
            ]
            for n in range(2):
                nc.sync.dma_start(
                    v_aug_bufs[n][:, :, 128:130],
                    ones_d[:].rearrange("p (t c) -> p t c", t=CT),
                )

            for i in range(NT):
                x16 = io1.tile([P, Ch], BF16, tag="xin")
                y16 = io1.tile([P, Ch], BF16, tag="yin")
                nc.gpsimd.dma_start(x16[:], inp_d[P * i : P * (i + 1), :])
                nc.gpsimd.dma_start(y16[:], y_d[P * i : P * (i + 1), :])

                tp = ps_tp.tile([P, 2 * CT, P], BF16, tag="tp")
                for t in range(CT):
                    nc.tensor.transpose(
                        tp[:, t, :], x16[:, P * t : P * (t + 1)], id16[:]
                    )
                for t in range(CT):
                    nc.tensor.transpose(
                        tp[:, CT + t, :], y16[:, P * t : P * (t + 1)], id16[:]
                    )
                xT8 = sb1.tile([P, CT, P], F8, tag="xT8")
                nc.scalar.copy(xT8[:], tp[:, 0:CT, :])
                nc.vector.tensor_copy(yT8[:, :, i, :], tp[:, CT : 2 * CT, :])

                kpre = ps_k.tile([P, Ch], F32, tag="kpre")
                for g in range(2):
                    nc.tensor.matmul(
                        kpre[:],
                        xT8[:, 2 * g : 2 * g + 2, :],
                        wk8[:, g, :, :],
                        start=(g == 0),
                        stop=False,
                        perf_mode=DR,
                    )
                for g in range(2):
                    nc.tensor.matmul(
                        kpre[:],
                        yT8[:, 2 * g : 2 * g + 2, i, :],
                        wk8[:, g, :, :],
                        start=False,
                        stop=(g == 1),
                        perf_mode=DR,
                    )
                khat = sb1.tile([P, Ch], BF16, tag="khat")
                nc.scalar.activation(khat[:], kpre[:], AF.Exp)

                vpre = ps_v.tile([P, Ch], F32, tag="vpre")
                for g in range(2):
                    nc.tensor.matmul(
                        vpre[:],
                        xT8[:, 2 * g : 2 * g + 2, :],
                        wv8[:, g, :, :],
                        start=(g == 0),
                        stop=(g == 1),
                        perf_mode=DR,
                    )
                v_aug = v_aug_bufs[i % 2]
                nc.vector.tensor_copy(
                    v_aug[:, :, 0:128],
                    vpre[:].rearrange("p (t q) -> p t q", t=CT),
                )

                for t in range(CT):
                    nc.tensor.matmul(
                        s_acc[t][:],
                        khat[:, P * t : P * (t + 1)],
                        v_aug[:, t, :],
                        start=(i == 0),
                        stop=(i == NT - 1),
                    )

            # ------------- epilogue: ctx = S * zkinv + bv ------------------
            for t in range(CT):
                nc.vector.reciprocal(zkinv[:, t : t + 1], s_acc[t][:, 128:129])
            for t in range(CT):
                nc.vector.tensor_copy(ctxR[:, t, :], bvb[:, P * t : P * (t + 1)])
                for blk in range(2):
                    p0 = 64 * blk
                    nc.vector.scalar_tensor_tensor(
                        ctxR[p0 : p0 + 64, t, p0 : p0 + 64],
                        s_acc[t][p0 : p0 + 64, p0 : p0 + 64],
                        zkinv[p0 : p0 + 64, t : t + 1],
                        bvb[p0 : p0 + 64, P * t + p0 : P * t + p0 + 64],
                        op0=mybir.AluOpType.mult,
                        op1=mybir.AluOpType.add,
                    )

        # ---------------- pass 2: q softmax, attend, reproject ---------------
        with (
            tc.tile_pool(name="io2", bufs=3) as io2,
            tc.tile_pool(name="sb2", bufs=2) as sb2,
            tc.tile_pool(name="ps_q", bufs=2, space="PSUM") as ps_q,
            tc.tile_pool(name="ps_dd", bufs=2, space="PSUM") as ps_dd,
            tc.tile_pool(name="ps_n", bufs=2, space="PSUM") as ps_n,
            tc.tile_pool(name="ps_o", bufs=2, space="PSUM") as ps_o,
        ):
            for gg in range(NG):
                j0 = GRP * gg
                u = sb2.tile([P, CT, Ch], BF16, tag="u")
                dden = ps_dd.tile([P, Ch], F32, tag="dd")
                for kb in range(CT):
                    qk = ps_q.tile([P, Ch], F32, tag="qk")
                    for g in range(2):
                        nc.tensor.matmul(
                            qk[:],
                            wq8[:, g, :, kb, :],
                            yT8[:, 2 * g : 2 * g + 2, j0 : j0 + GRP, :],
                            start=(g == 0),
                            stop=(g == 1),
                            perf_mode=DR,
                        )
                    nc.scalar.activation(
                        u[:, kb, :], qk[:], AF.Exp, bias=bq_col[:, kb : kb + 1]
                    )
                    nc.tensor.matmul(
                        dden[0:H, :],
                        sel_den[:, kb, :],
                        u[:, kb, :],
                        start=(kb == 0),
                        stop=(kb == CT - 1),
                    )
                deninv = sb2.tile([H, Ch], BF16, tag="dinv")
                with nc.allow_low_precision(reason="deninv rounded to bf16"):
                    nc.vector.reciprocal(deninv[:], dden[0:H, :])

                att = sb2.tile([P, CT, Ch], BF16, tag="att")
                for t in range(CT):
                    db = ps_dd.tile([P, Ch], F32, tag="dd")
                    nc.tensor.matmul(
                        db[:], selT[:, t, :], deninv[:], start=True, stop=True
                    )
                    dbs = sb2.tile([P, Ch], BF16, tag="dbs")
                    nc.scalar.copy(dbs[:], db[:])
                    num = ps_n.tile([P, Ch], F32, tag="num")
                    nc.tensor.matmul(
                        num[:], ctxR[:, t, :], u[:, t, :], start=True, stop=True
                    )
                    nc.vector.tensor_mul(att[:, t, :], num[:], dbs[:])

                for j in range(GRP):
                    i = j0 + j
                    opre = ps_o.tile([P, Ch], F32, tag="opre")
                    for t in range(CT):
                        nc.tensor.matmul(
                            opre[:],
                            att[:, t, P * j : P * (j + 1)],
                            wr16[:, t, :],
                            start=(t == 0),
                            stop=(t == CT - 1),
                        )
                    o_sb = io2.tile([P, Ch], F32, tag="osb")
                    nc.vector.tensor_add(o_sb[:], opre[:], brb[:])
                    nc.sync.dma_start(out_d[P * i : P * (i + 1), :], o_sb[:])

    nc.finalize()
    return nc


def _host_consts(Wk, bk, Wq, bq, Wv, bv, Wr, br):
    def w8(w):
        # [p, g, i, o] = W[256g + 128i + p, o]
        return np.ascontiguousarray(
            w.reshape(2, 2, P, Ch).transpose(2, 0, 1, 3).reshape(P, 4 * Ch)
        ).astype(F8_NP)

    # wq8: [p, g, i, kb, m] = Wq[256g + 128i + p, 128kb + m]
    wq8 = np.ascontiguousarray(
        Wq.reshape(2, 2, P, CT, P).transpose(2, 0, 1, 3, 4).reshape(P, 4 * Ch)
    ).astype(F8_NP)
    wr16 = np.ascontiguousarray(
        Wr.reshape(CT, P, Ch).transpose(1, 0, 2).reshape(P, CT * Ch)
    ).astype(BF16_NP)
    bq_col = np.ascontiguousarray(bq.reshape(CT, P).T).astype(np.float32)

    sel_den = np.zeros((P, CT, H), np.float32)
    for kb in range(CT):
        sel_den[0:64, kb, 2 * kb] = 1.0
        sel_den[64:128, kb, 2 * kb + 1] = 1.0

    selT_bc = np.zeros((H, CT, P), np.float32)
    for t in range(CT):
        selT_bc[2 * t, t, 0:64] = 1.0
        selT_bc[2 * t + 1, t, 64:128] = 1.0

    bvb = np.zeros((P, Ch), np.float32)
    for t in range(CT):
        for blk in range(2):
            p0 = 64 * blk
            c0 = P * t + p0
            bvb[p0 : p0 + 64, c0 : c0 + 64] = bv[None, c0 : c0 + 64]

    return {
        "wk8": w8(Wk),
        "wv8": w8(Wv),
        "wq8": wq8,
        "wr16": wr16,
        "bq_col": bq_col,
        "sel_den": sel_den.reshape(P, CT * H).astype(BF16_NP),
        "selT_bc": np.ascontiguousarray(selT_bc.reshape(H, CT * P)).astype(BF16_NP),
        "bv_blk": bvb,
        "br_bcast": np.ascontiguousarray(np.tile(br[None, :], (P, 1))).astype(
            np.float32
        ),
        "ident16": np.eye(P).astype(BF16_NP),
        "ones_col": np.ones((P, CT * 2), BF16_NP),
    }


_NC_CACHE = {}


def _get_nc():
    if "nc" not in _NC_CACHE:
        _NC_CACHE["nc"] = build_nc()
    return _NC_CACHE["nc"]


def kernel(input_, y, Wk, bk, Wq, bq, Wv, bv, Wr, br, _trace=False, _tmpdir=None):
    input_ = np.asarray(input_, np.float32)
    y = np.asarray(y, np.float32)
    consts = _host_consts(
        np.asarray(Wk, np.float32), np.asarray(bk, np.float32),
        np.asarray(Wq, np.float32), np.asarray(bq, np.float32),
        np.asarray(Wv, np.float32), np.asarray(bv, np.float32),
        np.asarray(Wr, np.float32), np.asarray(br, np.float32),
    )
    nc = _get_nc()
    in_maps = [
        {
            "input_": np.ascontiguousarray(input_[i]),
            "y": np.ascontiguousarray(y[i]),
            **consts,
        }
        for i in range(B)
    ]
    res = run_bass_kernel_spmd(
        nc, in_maps, core_ids=list(range(B)), trace=_trace, tmpdir=_tmpdir
    )
    out = np.stack([res.results[i]["out"] for i in range(B)], axis=0)
    if _trace:
        return out, res
    return out
